# revision 12
# baseline (speedup 1.0000x reference)
"""Low-pass FFT filtering kernel for Trainium2 (8 NeuronCores), v2.

Math: per (batch b, channel i), X = x[b,:,:,i] (256x256):
    out_i = P @ X_i + X_i @ P,  P = W @ W.T,  W [256, 31] orthonormal
    {1/sqrt(n), sqrt(2/n)cos(2pi k t/n), -sqrt(2/n)sin(2pi k t/n)}_{k=1..15}.

v2 vs v1: the host sends ONE fp16 copy of x per core (channel-major
[m, (i, n)]); the transposed layout needed for the X_i @ P term is
produced on-device by the DMA XBAR transpose (SBUF->SBUF, no HBM
traffic), cutting HBM bytes/core from ~13.7MB to ~8.8MB. C (=W^T X) and
D (=(X W)^T) land in one merged SBUF tile LR [126, cw]:
  partitions 0..30  Wt const | 31 zero | 32..62 D (63: 0)  ("L" = lhsT)
  partitions 64..94 C        | 95 zero | 96..126 Wt         ("R" = rhs)
so one [64, 512] ACT copy moves C and D together, and the phase-2
matmul is a single fused K=63 pass per (channel, m-half):
  p2 = W @ C_i + D_ij^T @ W^T  (both terms in one PSUM accumulation).
Constants are loaded once into a ring of 3 LR buffers instead of
per-chunk. Outputs ride the SP ring tail + SWDGE; transposes + consts
ride the ACT ring. Sharding: batch b -> core b (no communication).
"""

import os
import sys
import types

import numpy as np

import concourse.bass as bass
import concourse.bacc as bacc
import concourse.tile as tile
from concourse import mybir
from concourse.bass_utils import run_bass_kernel_spmd

B, M, N, I = 8, 256, 256, 32
KMAX = 16           # modes kept: 0..15
R = 2 * KMAX - 1    # 31 real basis vectors
FREE = I * N        # 8192
NCH = 8             # chunks
CW = FREE // NCH    # 1024 cols per chunk = 4 channels
CHC = I // NCH      # channels per chunk
NLR = 3             # LR buffer ring depth
F32 = mybir.dt.float32
F16 = mybir.dt.float16
NPDT = np.float16

LAST_RESULTS = None  # BassKernelResults of the most recent run (for test.py)


def _ensure_ntff_hook():
    """Provide antenv.axon_hooks if the image lacks it, so trace=True works."""
    try:
        from antenv.axon_hooks import get_axon_ntff_profile_hook  # noqa: F401
        return
    except ImportError:
        pass
    try:
        from trn_agent_boot.trn_boot import _ntff_profile_via_ctypes
        hook = _ntff_profile_via_ctypes("/opt/axon/libaxon_pjrt.so")
    except Exception:
        hook = None
    mod = types.ModuleType("antenv.axon_hooks")
    _state = {"hook": hook}
    mod.get_axon_ntff_profile_hook = lambda: _state["hook"]
    mod.set_axon_ntff_profile_hook = lambda h: _state.update(hook=h)
    sys.modules["antenv.axon_hooks"] = mod
    try:
        import antenv
        antenv.axon_hooks = mod
    except ImportError:
        pass


def _basis():
    t = np.arange(N)
    cols = [np.ones(N) / np.sqrt(N)]
    for k in range(1, KMAX):
        cols.append(np.sqrt(2.0 / N) * np.cos(2 * np.pi * k * t / N))
        cols.append(-np.sqrt(2.0 / N) * np.sin(2 * np.pi * k * t / N))
    return np.stack(cols, axis=1).astype(np.float32)  # [256, 31]


def _build_nc():
    nc = bacc.Bacc("TRN2", target_bir_lowering=False, debug=False,
                   enable_asserts=False, num_devices=8)

    xc = nc.declare_dram_parameter("xc", [M, FREE], F16, isOutput=False)
    w2 = nc.declare_dram_parameter("w2", [128, 64], F16, isOutput=False)
    wzl = nc.declare_dram_parameter("wzl", [32, CW], F16, isOutput=False)
    wzr = nc.declare_dram_parameter("wzr", [31, CW], F16, isOutput=False)
    out = nc.declare_dram_parameter("out", [M, FREE], F16, isOutput=True)

    with tile.TileContext(nc) as tc:
        with (
            tc.tile_pool(name="const", bufs=1) as constp,
            tc.tile_pool(name="xin", bufs=4) as xinp,
            tc.tile_pool(name="tt", bufs=4) as ttp,
            tc.tile_pool(name="oput", bufs=3) as outp,
            tc.tile_pool(name="pc", bufs=1, space=bass.MemorySpace.PSUM) as pcp,
            tc.tile_pool(name="pd", bufs=1, space=bass.MemorySpace.PSUM) as pdp,
            tc.tile_pool(name="p2", bufs=2, space=bass.MemorySpace.PSUM) as p2p,
        ):
            # consts ride SWDGE so the ACT ring is free for transposes
            w2sb = constp.tile([128, 64], F16)
            nc.gpsimd.dma_start(out=w2sb[:], in_=w2[:])
            # rings of L/R buffers; const rows filled once per buffer
            lgs, rgs = [], []
            for r_ in range(NLR):
                lg_t = constp.tile([64, CW], F16, name=f"lg{r_}")
                nc.gpsimd.dma_start(out=lg_t[0:32, :], in_=wzl[:])
                lgs.append(lg_t)
                rg_t = constp.tile([63, CW], F16, name=f"rg{r_}")
                nc.gpsimd.dma_start(out=rg_t[32:63, :], in_=wzr[:])
                rgs.append(rg_t)

            # Phase A: all input + transpose dispatches up front so no
            # per-chunk compute dependency ever blocks an engine's DMA queue.
            # Input of chunk g rides ring g%2, its transpose the other ring.
            # The XBAR is shared non-reentrant hardware: two DMA
            # transposes must NEVER be in flight concurrently, so they all
            # ride ONE ring (SP / Sync engine, which is otherwise idle).
            # Inputs ride the ACT ring.
            xins, tts = [], []
            for g in range(NCH):
                gsl = slice(g * CW, (g + 1) * CW)
                xin = xinp.tile([128, 2, CW], F16, tag="xin")
                nc.scalar.dma_start(
                    out=xin[:],
                    in_=xc[:, gsl].rearrange("(h p) c -> p h c", h=2))
                # XBAR transpose (SBUF->SBUF) of the whole chunk:
                # tt[n', gg, q] = xin[q, gg//8, (gg%8)*128 + n']
                # col (h*1024 + il*256 + n) -> gg = h*8 + 2*il + n//128
                tt = ttp.tile([128, 2 * CW // 128, 128], F16, tag="tt")
                nc.sync.dma_start(out=tt[:], in_=xin[:], transpose=True)
                xins.append(xin)
                tts.append(tt)

            # Phase B: software-pipelined PE stream. The Tensor engine is
            # in-order, so chunk g's C matmuls (which need only xin) are
            # emitted LA chunks ahead of its D/phase-2 block (which waits
            # for the serialized XBAR transpose chain). PSUM pools are
            # split (pc: C, pd: D, p2: phase-2) so no cross-chunk PSUM
            # dependency cycles form.
            LA = 3
            outs_sp, outs_act = [], []

            def emit_c(g):
                xin = xins[g]
                pc = pcp.tile([32, CW], F32, tag="pc")
                for f in range(2):  # 512-col f-block = 2 channels
                    fsl = slice(f * 512, (f + 1) * 512)
                    # C = W^T X into rows 0..30 (31: zero col of w2)
                    nc.tensor.matmul(pc[:, fsl], w2sb[:, 0:32],
                                     xin[:, 0, fsl], start=True, stop=False)
                    nc.tensor.matmul(pc[:, fsl], w2sb[:, 32:64],
                                     xin[:, 1, fsl], start=False, stop=True)
                Rg = rgs[g % NLR]
                nc.vector.tensor_copy(Rg[0:32, :], pc[:])

            def emit_dp(g):
                gsl = slice(g * CW, (g + 1) * CW)
                Lg = lgs[g % NLR]
                Rg = rgs[g % NLR]
                # view [128, kh, x, 128] with gg = x*2 + kh, x = h*4 + il
                tt_r = tts[g][:].rearrange("p (x y) m -> p y x m", y=2)
                pd = pdp.tile([64, CW], F32, tag="pd")
                dvv = pd[32:64, :].rearrange("p (i m) -> p i m", i=CHC)
                # D = (X W)^T rows 0..30 (31: zero); cols (il, h, m')
                for f in range(2):
                    for h in range(2):
                        dout = dvv[:, 2 * f:2 * f + 2, h * 128:(h + 1) * 128]
                        for kh in range(2):
                            x0 = h * 4 + 2 * f
                            nc.tensor.matmul(
                                dout, w2sb[:, kh * 32:(kh + 1) * 32],
                                tt_r[:, kh, x0:x0 + 2, :],
                                start=(kh == 0), stop=(kh == 1))
                nc.scalar.copy(Lg[32:64, :], pd[32:64, :])

                oo = outp.tile([128, 2, CW], F16, tag="oo")
                for j in range(2):  # m-half of the output
                    p2 = p2p.tile([128, CW], F32, tag="p2")
                    for ip in range(CHC):
                        c0 = ip * 256 + j * 128
                        nc.tensor.matmul(p2[:, ip * 256:(ip + 1) * 256],
                                         Lg[0:63, c0:c0 + 128],
                                         Rg[0:63, ip * 256:(ip + 1) * 256],
                                         start=True, stop=True)
                    nc.vector.tensor_copy(oo[:, j, :], p2[:])

                dst = out[:, gsl].rearrange("(j p) c -> p j c", j=2)
                if g >= NCH - 2:
                    # last chunks: SWDGE so they don't queue behind ring tails
                    nc.gpsimd.dma_start(out=dst, in_=oo[:])
                elif g >= NCH - 4:
                    outs_sp.append((dst, oo))
                else:
                    outs_act.append((dst, oo))

            for g in range(min(LA, NCH)):
                emit_c(g)
            for g in range(NCH):
                emit_dp(g)
                if g + LA < NCH:
                    emit_c(g + LA)

            # ring-tail outputs: FIFO-after all phase-A traffic.
            # Early chunks go on the ACT tail (free after inputs, ~13us);
            # later chunks on the SP tail (free after the transposes).
            for dst, oo in outs_act:
                nc.scalar.dma_start(out=dst, in_=oo[:])
            for dst, oo in outs_sp:
                nc.sync.dma_start(out=dst, in_=oo[:])

    nc.finalize()
    return nc


_NC = None


def kernel(x: np.ndarray) -> np.ndarray:
    global _NC, LAST_RESULTS
    x = np.asarray(x)
    assert x.shape == (B, M, N, I), x.shape

    W = _basis().astype(NPDT)          # [256, 31]
    Wt = W.T.copy()                    # [31, 256]
    zcol = np.zeros((128, 1), NPDT)
    w2_np = np.concatenate([W[0:128, :], zcol, W[128:256, :], zcol],
                           axis=1)                                # [128, 64]
    wtile = np.tile(Wt, (1, CHC))                                 # [31, CW]
    zrow = np.zeros((1, CW), NPDT)
    wzl_np = np.concatenate([wtile, zrow], axis=0)                # [32, CW]
    wzr_np = wtile                                                # [31, CW]

    if _NC is None:
        _NC = _build_nc()

    xq = np.asarray(x, dtype=NPDT)
    in_maps = []
    for b in range(B):
        xcm = np.ascontiguousarray(xq[b].transpose(0, 2, 1)).reshape(M, FREE)
        in_maps.append({
            "xc": xcm, "w2": w2_np, "wzl": wzl_np, "wzr": wzr_np,
        })

    trace = bool(int(os.environ.get("KERNEL_TRACE", "0")))
    if trace:
        _ensure_ntff_hook()
    last_err = None
    for attempt in range(3):
        try:
            LAST_RESULTS = run_bass_kernel_spmd(_NC, in_maps, list(range(B)),
                                                trace=trace and attempt == 0)
            break
        except Exception as e:  # rare transient NRT_EXEC_UNIT_UNRECOVERABLE
            last_err = e
            import time as _time
            _time.sleep(2.0)
            try:
                import jax
                jax.clear_caches()
                jax.extend.backend.clear_backends()
            except Exception:
                pass
    else:
        raise last_err

    out = np.empty((B, M, N, I), np.float32)
    for b in range(B):
        dev = LAST_RESULTS.results[b]["out"].astype(np.float32).reshape(M, I, N)
        out[b] = dev.transpose(0, 2, 1)
    return out


# revision 13
# speedup vs baseline: 1.3075x; 1.3075x over previous
"""Low-pass FFT filtering kernel for Trainium2 (8 NeuronCores), v2.

Math: per (batch b, channel i), X = x[b,:,:,i] (256x256):
    out_i = P @ X_i + X_i @ P,  P = W @ W.T,  W [256, 31] orthonormal
    {1/sqrt(n), sqrt(2/n)cos(2pi k t/n), -sqrt(2/n)sin(2pi k t/n)}_{k=1..15}.

v2 vs v1: the host sends ONE fp16 copy of x per core (channel-major
[m, (i, n)]); the transposed layout needed for the X_i @ P term is
produced on-device by the DMA XBAR transpose (SBUF->SBUF, no HBM
traffic), cutting HBM bytes/core from ~13.7MB to ~8.8MB. C (=W^T X) and
D (=(X W)^T) land in one merged SBUF tile LR [126, cw]:
  partitions 0..30  Wt const | 31 zero | 32..62 D (63: 0)  ("L" = lhsT)
  partitions 64..94 C        | 95 zero | 96..126 Wt         ("R" = rhs)
so one [64, 512] ACT copy moves C and D together, and the phase-2
matmul is a single fused K=63 pass per (channel, m-half):
  p2 = W @ C_i + D_ij^T @ W^T  (both terms in one PSUM accumulation).
Constants are loaded once into a ring of 3 LR buffers instead of
per-chunk. Outputs ride the SP ring tail + SWDGE; transposes + consts
ride the ACT ring. Sharding: batch b -> core b (no communication).
"""

import os
import sys
import types

import numpy as np

import concourse.bass as bass
import concourse.bacc as bacc
import concourse.tile as tile
from concourse import mybir
from concourse.bass_utils import run_bass_kernel_spmd

B, M, N, I = 8, 256, 256, 32
KMAX = 16           # modes kept: 0..15
R = 2 * KMAX - 1    # 31 real basis vectors
FREE = I * N        # 8192
NCH = 8             # chunks
CW = FREE // NCH    # 1024 cols per chunk = 4 channels
CHC = I // NCH      # channels per chunk
NLR = 3             # LR buffer ring depth
F32 = mybir.dt.float32
F16 = mybir.dt.float16
NPDT = np.float16

LAST_RESULTS = None  # BassKernelResults of the most recent run (for test.py)


def _ensure_ntff_hook():
    """Provide antenv.axon_hooks if the image lacks it, so trace=True works."""
    try:
        from antenv.axon_hooks import get_axon_ntff_profile_hook  # noqa: F401
        return
    except ImportError:
        pass
    try:
        from trn_agent_boot.trn_boot import _ntff_profile_via_ctypes
        hook = _ntff_profile_via_ctypes("/opt/axon/libaxon_pjrt.so")
    except Exception:
        hook = None
    mod = types.ModuleType("antenv.axon_hooks")
    _state = {"hook": hook}
    mod.get_axon_ntff_profile_hook = lambda: _state["hook"]
    mod.set_axon_ntff_profile_hook = lambda h: _state.update(hook=h)
    sys.modules["antenv.axon_hooks"] = mod
    try:
        import antenv
        antenv.axon_hooks = mod
    except ImportError:
        pass


def _basis():
    t = np.arange(N)
    cols = [np.ones(N) / np.sqrt(N)]
    for k in range(1, KMAX):
        cols.append(np.sqrt(2.0 / N) * np.cos(2 * np.pi * k * t / N))
        cols.append(-np.sqrt(2.0 / N) * np.sin(2 * np.pi * k * t / N))
    return np.stack(cols, axis=1).astype(np.float32)  # [256, 31]


def _build_nc():
    nc = bacc.Bacc("TRN2", target_bir_lowering=False, debug=False,
                   enable_asserts=False, num_devices=8)

    xc = nc.declare_dram_parameter("xc", [M, FREE], F16, isOutput=False)
    xt = nc.declare_dram_parameter("xt", [N, I * M], F16, isOutput=False)
    w2 = nc.declare_dram_parameter("w2", [128, 64], F16, isOutput=False)
    wzl = nc.declare_dram_parameter("wzl", [32, CW], F16, isOutput=False)
    wzr = nc.declare_dram_parameter("wzr", [31, CW], F16, isOutput=False)
    out = nc.declare_dram_parameter("out", [M, FREE], F16, isOutput=True)

    with tile.TileContext(nc) as tc:
        with (
            tc.tile_pool(name="const", bufs=1) as constp,
            tc.tile_pool(name="xin", bufs=4) as xinp,
            tc.tile_pool(name="tt", bufs=4) as ttp,
            tc.tile_pool(name="oput", bufs=3) as outp,
            tc.tile_pool(name="pc", bufs=1, space=bass.MemorySpace.PSUM) as pcp,
            tc.tile_pool(name="pd", bufs=1, space=bass.MemorySpace.PSUM) as pdp,
            tc.tile_pool(name="p2", bufs=2, space=bass.MemorySpace.PSUM) as p2p,
        ):
            # consts ride SWDGE so the ACT ring is free for transposes
            w2sb = constp.tile([128, 64], F16)
            nc.gpsimd.dma_start(out=w2sb[:], in_=w2[:])
            # rings of L/R buffers; const rows filled once per buffer
            lgs, rgs = [], []
            for r_ in range(NLR):
                lg_t = constp.tile([64, CW], F16, name=f"lg{r_}")
                nc.gpsimd.dma_start(out=lg_t[0:32, :], in_=wzl[:])
                lgs.append(lg_t)
                rg_t = constp.tile([63, CW], F16, name=f"rg{r_}")
                nc.gpsimd.dma_start(out=rg_t[32:63, :], in_=wzr[:])
                rgs.append(rg_t)

            # Phase A: all input + transpose dispatches up front so no
            # per-chunk compute dependency ever blocks an engine's DMA queue.
            # Input of chunk g rides ring g%2, its transpose the other ring.
            # Phase A: stream both host-prepared layouts, xc on the ACT
            # ring and xt (host-transposed) on the SP ring.
            xins, xtins = [], []
            for g in range(NCH):
                gsl = slice(g * CW, (g + 1) * CW)
                xin = xinp.tile([128, 2, CW], F16, tag="xin")
                nc.scalar.dma_start(
                    out=xin[:],
                    in_=xc[:, gsl].rearrange("(h p) c -> p h c", h=2))
                xtin = ttp.tile([128, 2, CW], F16, tag="xt")
                nc.sync.dma_start(
                    out=xtin[:],
                    in_=xt[:, gsl].rearrange("(h p) c -> p h c", h=2))
                xins.append(xin)
                xtins.append(xtin)

            # Phase B: software-pipelined PE stream. The Tensor engine is
            # in-order, so chunk g's C matmuls (which need only xin) are
            # emitted LA chunks ahead of its D/phase-2 block (which waits
            # for the serialized XBAR transpose chain). PSUM pools are
            # split (pc: C, pd: D, p2: phase-2) so no cross-chunk PSUM
            # dependency cycles form.
            LA = 1
            outs_sp, outs_act = [], []

            def emit_c(g):
                xin = xins[g]
                pc = pcp.tile([32, CW], F32, tag="pc")
                for f in range(2):  # 512-col f-block = 2 channels
                    fsl = slice(f * 512, (f + 1) * 512)
                    # C = W^T X into rows 0..30 (31: zero col of w2)
                    nc.tensor.matmul(pc[:, fsl], w2sb[:, 0:32],
                                     xin[:, 0, fsl], start=True, stop=False)
                    nc.tensor.matmul(pc[:, fsl], w2sb[:, 32:64],
                                     xin[:, 1, fsl], start=False, stop=True)
                Rg = rgs[g % NLR]
                nc.vector.tensor_copy(Rg[0:32, :], pc[:])

            def emit_dp(g):
                gsl = slice(g * CW, (g + 1) * CW)
                Lg = lgs[g % NLR]
                Rg = rgs[g % NLR]
                xtin = xtins[g]
                pd = pdp.tile([64, CW], F32, tag="pd")
                # D = W^T X^T into rows 32..62 (63: zero col of w2);
                # xt cols are (il, m) so D lands in Lg layout directly
                for f in range(2):
                    fsl = slice(f * 512, (f + 1) * 512)
                    nc.tensor.matmul(pd[32:64, fsl], w2sb[:, 0:32],
                                     xtin[:, 0, fsl], start=True, stop=False)
                    nc.tensor.matmul(pd[32:64, fsl], w2sb[:, 32:64],
                                     xtin[:, 1, fsl], start=False, stop=True)
                nc.scalar.copy(Lg[32:64, :], pd[32:64, :])

                oo = outp.tile([128, 2, CW], F16, tag="oo")
                for j in range(2):  # m-half of the output
                    p2 = p2p.tile([128, CW], F32, tag="p2")
                    for ip in range(CHC):
                        c0 = ip * 256 + j * 128
                        nc.tensor.matmul(p2[:, ip * 256:(ip + 1) * 256],
                                         Lg[0:63, c0:c0 + 128],
                                         Rg[0:63, ip * 256:(ip + 1) * 256],
                                         start=True, stop=True)
                    nc.vector.tensor_copy(oo[:, j, :], p2[:])

                dst = out[:, gsl].rearrange("(j p) c -> p j c", j=2)
                if g >= NCH - 2:
                    # last chunks: SWDGE so they don't queue behind ring tails
                    nc.gpsimd.dma_start(out=dst, in_=oo[:])
                elif g >= NCH - 4:
                    outs_sp.append((dst, oo))
                else:
                    outs_act.append((dst, oo))

            for g in range(min(LA, NCH)):
                emit_c(g)
            for g in range(NCH):
                emit_dp(g)
                if g + LA < NCH:
                    emit_c(g + LA)

            # ring-tail outputs: FIFO-after all phase-A traffic.
            # Early chunks go on the ACT tail (free after inputs, ~13us);
            # later chunks on the SP tail (free after the transposes).
            for dst, oo in outs_act:
                nc.scalar.dma_start(out=dst, in_=oo[:])
            for dst, oo in outs_sp:
                nc.sync.dma_start(out=dst, in_=oo[:])

    nc.finalize()
    return nc


_NC = None


def kernel(x: np.ndarray) -> np.ndarray:
    global _NC, LAST_RESULTS
    x = np.asarray(x)
    assert x.shape == (B, M, N, I), x.shape

    W = _basis().astype(NPDT)          # [256, 31]
    Wt = W.T.copy()                    # [31, 256]
    zcol = np.zeros((128, 1), NPDT)
    w2_np = np.concatenate([W[0:128, :], zcol, W[128:256, :], zcol],
                           axis=1)                                # [128, 64]
    wtile = np.tile(Wt, (1, CHC))                                 # [31, CW]
    zrow = np.zeros((1, CW), NPDT)
    wzl_np = np.concatenate([wtile, zrow], axis=0)                # [32, CW]
    wzr_np = wtile                                                # [31, CW]

    if _NC is None:
        _NC = _build_nc()

    xq = np.asarray(x, dtype=NPDT)
    in_maps = []
    for b in range(B):
        xcm = np.ascontiguousarray(xq[b].transpose(0, 2, 1)).reshape(M, FREE)
        xtm = np.ascontiguousarray(xq[b].transpose(1, 2, 0)).reshape(N, I * M)
        in_maps.append({
            "xc": xcm, "xt": xtm, "w2": w2_np, "wzl": wzl_np, "wzr": wzr_np,
        })

    trace = bool(int(os.environ.get("KERNEL_TRACE", "0")))
    if trace:
        _ensure_ntff_hook()
    last_err = None
    for attempt in range(3):
        try:
            LAST_RESULTS = run_bass_kernel_spmd(_NC, in_maps, list(range(B)),
                                                trace=trace and attempt == 0)
            break
        except Exception as e:  # rare transient NRT_EXEC_UNIT_UNRECOVERABLE
            last_err = e
            import time as _time
            _time.sleep(2.0)
            try:
                import jax
                jax.clear_caches()
                jax.extend.backend.clear_backends()
            except Exception:
                pass
    else:
        raise last_err

    out = np.empty((B, M, N, I), np.float32)
    for b in range(B):
        dev = LAST_RESULTS.results[b]["out"].astype(np.float32).reshape(M, I, N)
        out[b] = dev.transpose(0, 2, 1)
    return out


# revision 14
# speedup vs baseline: 1.6478x; 1.2602x over previous
"""Low-pass FFT filtering kernel for Trainium2 (8 NeuronCores), v2.

Math: per (batch b, channel i), X = x[b,:,:,i] (256x256):
    out_i = P @ X_i + X_i @ P,  P = W @ W.T,  W [256, 31] orthonormal
    {1/sqrt(n), sqrt(2/n)cos(2pi k t/n), -sqrt(2/n)sin(2pi k t/n)}_{k=1..15}.

v2 vs v1: the host sends ONE fp16 copy of x per core (channel-major
[m, (i, n)]); the transposed layout needed for the X_i @ P term is
produced on-device by the DMA XBAR transpose (SBUF->SBUF, no HBM
traffic), cutting HBM bytes/core from ~13.7MB to ~8.8MB. C (=W^T X) and
D (=(X W)^T) land in one merged SBUF tile LR [126, cw]:
  partitions 0..30  Wt const | 31 zero | 32..62 D (63: 0)  ("L" = lhsT)
  partitions 64..94 C        | 95 zero | 96..126 Wt         ("R" = rhs)
so one [64, 512] ACT copy moves C and D together, and the phase-2
matmul is a single fused K=63 pass per (channel, m-half):
  p2 = W @ C_i + D_ij^T @ W^T  (both terms in one PSUM accumulation).
Constants are loaded once into a ring of 3 LR buffers instead of
per-chunk. Outputs ride the SP ring tail + SWDGE; transposes + consts
ride the ACT ring. Sharding: batch b -> core b (no communication).
"""

import os
import sys
import types

import numpy as np

import concourse.bass as bass
import concourse.bacc as bacc
import concourse.tile as tile
from concourse import mybir
from concourse.bass_utils import run_bass_kernel_spmd

B, M, N, I = 8, 256, 256, 32
KMAX = 16           # modes kept: 0..15
R = 2 * KMAX - 1    # 31 real basis vectors
FREE = I * N        # 8192
NCH = 8             # chunks
CW = FREE // NCH    # 1024 cols per chunk = 4 channels
CHC = I // NCH      # channels per chunk
NLR = 3             # LR buffer ring depth
F32 = mybir.dt.float32
F16 = mybir.dt.float16
NPDT = np.float16

LAST_RESULTS = None  # BassKernelResults of the most recent run (for test.py)


def _ensure_ntff_hook():
    """Provide antenv.axon_hooks if the image lacks it, so trace=True works."""
    try:
        from antenv.axon_hooks import get_axon_ntff_profile_hook  # noqa: F401
        return
    except ImportError:
        pass
    try:
        from trn_agent_boot.trn_boot import _ntff_profile_via_ctypes
        hook = _ntff_profile_via_ctypes("/opt/axon/libaxon_pjrt.so")
    except Exception:
        hook = None
    mod = types.ModuleType("antenv.axon_hooks")
    _state = {"hook": hook}
    mod.get_axon_ntff_profile_hook = lambda: _state["hook"]
    mod.set_axon_ntff_profile_hook = lambda h: _state.update(hook=h)
    sys.modules["antenv.axon_hooks"] = mod
    try:
        import antenv
        antenv.axon_hooks = mod
    except ImportError:
        pass


def _basis():
    t = np.arange(N)
    cols = [np.ones(N) / np.sqrt(N)]
    for k in range(1, KMAX):
        cols.append(np.sqrt(2.0 / N) * np.cos(2 * np.pi * k * t / N))
        cols.append(-np.sqrt(2.0 / N) * np.sin(2 * np.pi * k * t / N))
    return np.stack(cols, axis=1).astype(np.float32)  # [256, 31]


def _build_nc():
    nc = bacc.Bacc("TRN2", target_bir_lowering=False, debug=False,
                   enable_asserts=False, num_devices=8)

    xc = nc.declare_dram_parameter("xc", [M, FREE], F16, isOutput=False)
    xt = nc.declare_dram_parameter("xt", [N, I * M], F16, isOutput=False)
    w2 = nc.declare_dram_parameter("w2", [128, 64], F16, isOutput=False)
    wzl = nc.declare_dram_parameter("wzl", [32, CW], F16, isOutput=False)
    wzr = nc.declare_dram_parameter("wzr", [31, CW], F16, isOutput=False)
    out = nc.declare_dram_parameter("out", [M, FREE], F16, isOutput=True)

    with tile.TileContext(nc) as tc:
        with (
            tc.tile_pool(name="const", bufs=1) as constp,
            tc.tile_pool(name="xin", bufs=4) as xinp,
            tc.tile_pool(name="tt", bufs=4) as ttp,
            tc.tile_pool(name="oput", bufs=3) as outp,
            tc.tile_pool(name="pcd", bufs=2, space=bass.MemorySpace.PSUM) as pcdp,
            tc.tile_pool(name="p2", bufs=2, space=bass.MemorySpace.PSUM) as p2p,
        ):
            # consts ride SWDGE so the ACT ring is free for transposes
            w2sb = constp.tile([128, 64], F16)
            nc.gpsimd.dma_start(out=w2sb[:], in_=w2[:])
            # rings of L/R buffers; const rows filled once per buffer
            lgs, rgs = [], []
            for r_ in range(NLR):
                lg_t = constp.tile([64, CW], F16, name=f"lg{r_}")
                nc.gpsimd.dma_start(out=lg_t[0:32, :], in_=wzl[:])
                lgs.append(lg_t)
                rg_t = constp.tile([63, CW], F16, name=f"rg{r_}")
                nc.gpsimd.dma_start(out=rg_t[32:63, :], in_=wzr[:])
                rgs.append(rg_t)

            # Phase A: all input + transpose dispatches up front so no
            # per-chunk compute dependency ever blocks an engine's DMA queue.
            # Input of chunk g rides ring g%2, its transpose the other ring.
            # Phase A: stream both host-prepared layouts, xc on the ACT
            # ring and xt (host-transposed) on the SP ring.
            xins, xtins = [], []
            for g in range(NCH):
                gsl = slice(g * CW, (g + 1) * CW)
                xin = xinp.tile([128, 2, CW], F16, tag="xin")
                nc.scalar.dma_start(
                    out=xin[:],
                    in_=xc[:, gsl].rearrange("(h p) c -> p h c", h=2))
                xtin = ttp.tile([128, 2, CW], F16, tag="xt")
                nc.sync.dma_start(
                    out=xtin[:],
                    in_=xt[:, gsl].rearrange("(h p) c -> p h c", h=2))
                xins.append(xin)
                xtins.append(xtin)

            # Phase B: software-pipelined PE stream. The Tensor engine is
            # in-order, so chunk g's C matmuls (which need only xin) are
            # emitted LA chunks ahead of its D/phase-2 block (which waits
            # for the serialized XBAR transpose chain). PSUM pools are
            # split (pc: C, pd: D, p2: phase-2) so no cross-chunk PSUM
            # dependency cycles form.
            # PE stream: [C_g D_g P2_{g-2}] triples. Phase-2 of chunk g
            # runs two chunks late so the PSUM->SBUF copies (and their
            # semaphore latencies) complete in the shadow of other chunks'
            # matmuls and the PE never waits on the copy round trip.
            PD = 2
            outs_sp, outs_act = [], []

            def emit_cd(g):
                xin = xins[g]
                xtin = xtins[g]
                pcd = pcdp.tile([64, CW], F32, tag="pcd")
                for f in range(2):  # 512-col f-block = 2 channels
                    fsl = slice(f * 512, (f + 1) * 512)
                    # C = W^T X into rows 0..30 (31: zero col of w2)
                    nc.tensor.matmul(pcd[0:32, fsl], w2sb[:, 0:32],
                                     xin[:, 0, fsl], start=True, stop=False)
                    nc.tensor.matmul(pcd[0:32, fsl], w2sb[:, 32:64],
                                     xin[:, 1, fsl], start=False, stop=True)
                    # D = W^T X^T into rows 32..62 (63: zero col of w2);
                    # xt cols are (il, m) so D lands in Lg layout directly
                    nc.tensor.matmul(pcd[32:64, fsl], w2sb[:, 0:32],
                                     xtin[:, 0, fsl], start=True, stop=False)
                    nc.tensor.matmul(pcd[32:64, fsl], w2sb[:, 32:64],
                                     xtin[:, 1, fsl], start=False, stop=True)
                nc.vector.tensor_copy(rgs[g % NLR][0:32, :], pcd[0:32, :])
                nc.scalar.copy(lgs[g % NLR][32:64, :], pcd[32:64, :])

            def emit_p2(g):
                gsl = slice(g * CW, (g + 1) * CW)
                Lg = lgs[g % NLR]
                Rg = rgs[g % NLR]
                oo = outp.tile([128, 2, CW], F16, tag="oo")
                for j in range(2):  # m-half of the output
                    p2 = p2p.tile([128, CW], F32, tag="p2")
                    for ip in range(CHC):
                        c0 = ip * 256 + j * 128
                        nc.tensor.matmul(p2[:, ip * 256:(ip + 1) * 256],
                                         Lg[0:63, c0:c0 + 128],
                                         Rg[0:63, ip * 256:(ip + 1) * 256],
                                         start=True, stop=True)
                    nc.vector.tensor_copy(oo[:, j, :], p2[:])

                dst = out[:, gsl].rearrange("(j p) c -> p j c", j=2)
                if g >= NCH - 2:
                    # last chunks: SWDGE so they don't queue behind ring tails
                    nc.gpsimd.dma_start(out=dst, in_=oo[:])
                elif g >= NCH - 4:
                    outs_sp.append((dst, oo))
                else:
                    outs_act.append((dst, oo))

            for g in range(NCH):
                emit_cd(g)
                if g >= PD:
                    emit_p2(g - PD)
            for g in range(NCH - PD, NCH):
                emit_p2(g)

            # ring-tail outputs: FIFO-after all phase-A traffic.
            # Early chunks go on the ACT tail (free after inputs, ~13us);
            # later chunks on the SP tail (free after the transposes).
            for dst, oo in outs_act:
                nc.scalar.dma_start(out=dst, in_=oo[:])
            for dst, oo in outs_sp:
                nc.sync.dma_start(out=dst, in_=oo[:])

    nc.finalize()
    return nc


_NC = None


def kernel(x: np.ndarray) -> np.ndarray:
    global _NC, LAST_RESULTS
    x = np.asarray(x)
    assert x.shape == (B, M, N, I), x.shape

    W = _basis().astype(NPDT)          # [256, 31]
    Wt = W.T.copy()                    # [31, 256]
    zcol = np.zeros((128, 1), NPDT)
    w2_np = np.concatenate([W[0:128, :], zcol, W[128:256, :], zcol],
                           axis=1)                                # [128, 64]
    wtile = np.tile(Wt, (1, CHC))                                 # [31, CW]
    zrow = np.zeros((1, CW), NPDT)
    wzl_np = np.concatenate([wtile, zrow], axis=0)                # [32, CW]
    wzr_np = wtile                                                # [31, CW]

    if _NC is None:
        _NC = _build_nc()

    xq = np.asarray(x, dtype=NPDT)
    in_maps = []
    for b in range(B):
        xcm = np.ascontiguousarray(xq[b].transpose(0, 2, 1)).reshape(M, FREE)
        xtm = np.ascontiguousarray(xq[b].transpose(1, 2, 0)).reshape(N, I * M)
        in_maps.append({
            "xc": xcm, "xt": xtm, "w2": w2_np, "wzl": wzl_np, "wzr": wzr_np,
        })

    trace = bool(int(os.environ.get("KERNEL_TRACE", "0")))
    if trace:
        _ensure_ntff_hook()
    last_err = None
    for attempt in range(3):
        try:
            LAST_RESULTS = run_bass_kernel_spmd(_NC, in_maps, list(range(B)),
                                                trace=trace and attempt == 0)
            break
        except Exception as e:  # rare transient NRT_EXEC_UNIT_UNRECOVERABLE
            last_err = e
            import time as _time
            _time.sleep(2.0)
            try:
                import jax
                jax.clear_caches()
                jax.extend.backend.clear_backends()
            except Exception:
                pass
    else:
        raise last_err

    out = np.empty((B, M, N, I), np.float32)
    for b in range(B):
        dev = LAST_RESULTS.results[b]["out"].astype(np.float32).reshape(M, I, N)
        out[b] = dev.transpose(0, 2, 1)
    return out
